# revision 64
# baseline (speedup 1.0000x reference)
"""Block-sparse linear y = x @ W^T + b on 8 TRN2 NeuronCores.

Problem shape (hardcoded): x [8192, 4096] f32, weight [1024, 64, 64] f32
(64x64 blocks), bias [4096] f32, row_idx/col_idx [1024] int32 over a 64x64
block grid.

V11 strategy: data-parallel over tokens (1024/core). y^T = W x^T + b via
64x64 block matmuls packed 4-wide into PE-array quadrants (tile_position),
bf16 inputs with f32 PSUM accumulate. Two 512-token slab passes: pass s
runs entirely on x-slab s; weights stream on their own (sync) queue paced
at consumption by a small tile pool (descriptor-level reuse waits), re-
streamed per pass, so x keeps DMA priority in the prologue. x streams on
two queues (scalar/gpsimd) with fine 256KB chunks for slab 0 so compute
starts early. Each col-block c gets a host-chosen partition half (ki) +
position; halves balance the four (ki, mi=row-parity) quadrant loads and
positions follow pair-discovery order so early pairs complete on early
chunks and the 4-unit PSUM window rolls. Matmuls weave across the 4
quadrants globally with a multi-unit lookahead so chain imbalance never
idles the PE; ~4us of dummy matmuls pre-warm the HAM clock gate. Output
y^T bf16 on the sync queue (fast drain), bias added on eviction via
scalar activation + vector add, f32 cast on host.
"""

from collections import deque
from contextlib import ExitStack

import numpy as np
import ml_dtypes

import concourse.tile as tile
from concourse import bacc, mybir
from concourse.bass_utils import run_bass_kernel_spmd

BLK = 64
OUT_BLK = 64
IN_BLK = 64
D_IN = IN_BLK * BLK    # 4096
D_OUT = OUT_BLK * BLK  # 4096
N_CORES = 8
N_PAIRS = OUT_BLK // 2
TOK = 512              # tokens per pass (slab)
XCH = 8                # positions per x DMA chunk, slabs >= 1 (1MB)
XCH0 = 2               # fine chunks for slab-0 head (256KB): early start
N_FINE = 4             # slab-0 fine chunks (pos 0..XCH0*N_FINE-1); the
                       # rest of slab 0 uses XCH-wide chunks (better BW)
WGRP = 32              # weight slots per DMA group per ki half (512KB)
WBUFS = 4              # weight pool buffers: paces the stream to
                       # consumption via descriptor-level reuse waits
WEAVE_Q = 40           # refill quadrant queues to this many MMs
BF16 = ml_dtypes.bfloat16
QUADS = ((0, 0), (1, 1), (0, 1), (1, 0))


def _assign_cols(row_idx, col_idx):
    """Assign each col-block c a partition half (ki) and position so the
    four (ki, mi=row-parity) quadrant loads are balanced and each half
    holds exactly IN_BLK//2 cols."""
    n_ev = np.zeros(IN_BLK, dtype=np.int64)
    n_od = np.zeros(IN_BLK, dtype=np.int64)
    for r, c in zip(row_idx, col_idx):
        if int(r) % 2 == 0:
            n_ev[int(c)] += 1
        else:
            n_od[int(c)] += 1
    # iterate cols in discovery order (first pair that uses them) so each
    # early pair's cols split evenly across halves -> early units finish
    # on early x chunks; balance quadrant loads as the primary objective
    first_use = {}
    for r, c in zip(row_idx, col_idx):
        p = int(r) // 2
        if int(c) not in first_use or p < first_use[int(c)]:
            first_use[int(c)] = p
    order = sorted(range(IN_BLK),
                   key=lambda c: (first_use.get(c, N_PAIRS),
                                  -(n_ev[c] + n_od[c])))
    cap = IN_BLK // 2
    load = [[0, 0], [0, 0]]  # [ki][mi]
    cnt = [0, 0]
    half = [0] * IN_BLK
    for c in order:
        best, bestcost = None, None
        for k in (0, 1):
            if cnt[k] >= cap:
                continue
            cost = max(load[k][0] + n_ev[c], load[k][1] + n_od[c])
            if bestcost is None or cost < bestcost:
                best, bestcost = k, cost
        half[c] = best
        load[best][0] += n_ev[c]
        load[best][1] += n_od[c]
        cnt[best] += 1
    # positions in discovery order: cols of early pairs get low positions
    # so early units complete on early x chunks and the PSUM window rolls
    pos = [-1] * IN_BLK
    nxt = [0, 0]
    by_pair = [[] for _ in range(N_PAIRS)]
    for r, c in zip(row_idx, col_idx):
        by_pair[int(r) // 2].append(int(c))
    for p in range(N_PAIRS):
        for c in sorted(by_pair[p]):
            if pos[c] < 0:
                pos[c] = nxt[half[c]]
                nxt[half[c]] += 1
    for c in range(IN_BLK):
        if pos[c] < 0:
            pos[c] = nxt[half[c]]
            nxt[half[c]] += 1
    return half, pos


def _build_pairs(row_idx, col_idx, half, pos):
    """pairs[p] = list of (pos, c, mi, w, ki) sorted by pos (consumption
    order within the x stream)."""
    d = {}
    for i in range(len(row_idx)):
        d[(int(row_idx[i]), int(col_idx[i]))] = i
    pairs = [[] for _ in range(N_PAIRS)]
    for (r, c), w in d.items():
        pairs[r // 2].append((pos[c], c, r % 2, w, half[c]))
    for lst in pairs:
        lst.sort()
    return pairs


def _weave(pairs):
    """Yield ('mm', p, block) / ('evict', p) events for one pass, weaving
    MMs across the four quadrants globally. Blocks from a small window of
    units are interleaved so chain tails never idle a quadrant."""
    q = {qd: deque() for qd in QUADS}
    remaining = {}
    nxt = 0
    done_order = deque()

    def admit():
        nonlocal nxt
        p = nxt
        for blk in pairs[p]:
            _, c, mi, w, ki = blk
            q[(ki, mi)].append((p, blk))
        if pairs[p]:
            remaining[p] = len(pairs[p])
        done_order.append(p)
        nxt += 1

    while nxt < N_PAIRS or any(q.values()) or done_order:
        while nxt < N_PAIRS and (
                sum(len(v) for v in q.values()) < WEAVE_Q):
            admit()
        for qd in QUADS:
            if q[qd]:
                p, blk = q[qd].popleft()
                yield ("mm", p, blk)
                remaining[p] -= 1
                if remaining[p] == 0:
                    del remaining[p]
        while done_order and done_order[0] not in remaining:
            yield ("evict", done_order.popleft())


def _assign_slots(pairs):
    """Weight slot per block, in pass-0 emission order so weight groups
    stream in consumption order."""
    slot = {}
    cnt = [0, 0]
    for ev in _weave(pairs):
        if ev[0] != "mm":
            continue
        _, p, (_, c, mi, w, ki) = ev
        slot[w] = cnt[ki]
        cnt[ki] += 1
    n_groups = (max(cnt) + WGRP - 1) // WGRP
    return slot, max(n_groups, 1)


def _pack_host_arrays(weight, bias, pairs, slot, n_groups):
    wgrp = np.zeros((n_groups, 128, WGRP * BLK), dtype=BF16)
    bias_pk = np.zeros((128, N_PAIRS), dtype=np.float32)
    wT = np.ascontiguousarray(
        np.transpose(np.asarray(weight), (0, 2, 1))).astype(BF16)
    for p in range(N_PAIRS):
        bias_pk[:64, p] = bias[(2 * p) * BLK:(2 * p + 1) * BLK]
        bias_pk[64:, p] = bias[(2 * p + 1) * BLK:(2 * p + 2) * BLK]
        for (_, c, mi, w, ki) in pairs[p]:
            g, j = divmod(slot[w], WGRP)
            wgrp[g, ki * 64:(ki + 1) * 64, j * BLK:(j + 1) * BLK] = wT[w]
    return wgrp, bias_pk


def _slab0_chunks():
    """Slab-0 chunk spec: (start_pos, width) — N_FINE fine chunks for the
    earliest positions, then XCH-wide chunks for the bulk."""
    spec = [(i * XCH0, XCH0) for i in range(N_FINE)]
    p0 = N_FINE * XCH0
    while p0 < IN_BLK // 2:
        w = min(XCH, IN_BLK // 2 - p0)
        spec.append((p0, w))
        p0 += w
    return spec


def _build_kernel(pairs, slot, n_groups, ntok):
    assert ntok % TOK == 0
    n_sl = ntok // TOK
    ch0 = _slab0_chunks()
    n_fine = N_FINE
    n_bulk = len(ch0) - n_fine
    n_chunks = (IN_BLK // 2 + XCH - 1) // XCH
    sdt = mybir.dt.bfloat16
    f32 = mybir.dt.float32

    nc = bacc.Bacc("TRN2", target_bir_lowering=False, debug=False)
    xt0a_d = nc.dram_tensor("xt0a_pk", [n_fine, 128, XCH0 * TOK], sdt,
                            kind="ExternalInput").ap()
    xt0b_d = nc.dram_tensor("xt0b_pk", [n_bulk, 128, XCH * TOK], sdt,
                            kind="ExternalInput").ap()
    xt_d = nc.dram_tensor("xt_pk",
                          [max(n_sl - 1, 1), n_chunks, 128, XCH * TOK],
                          sdt, kind="ExternalInput").ap()
    wg_d = nc.dram_tensor("wgrp", [n_groups, 128, WGRP * BLK], sdt,
                          kind="ExternalInput").ap()
    bias_d = nc.dram_tensor("bias_pk", [128, N_PAIRS], f32,
                            kind="ExternalInput").ap()
    yt_d = nc.dram_tensor("yt", [D_OUT, ntok], sdt,
                          kind="ExternalOutput").ap()

    nmm = []
    for p in range(N_PAIRS):
        m = {qd: 0 for qd in QUADS}
        for (_, c, mi, w, ki) in pairs[p]:
            m[(ki, mi)] += 1
        nmm.append(m)

    with tile.TileContext(nc) as tc:
        with ExitStack() as ctx:
            xpool = ctx.enter_context(tc.tile_pool(name="xp", bufs=1))
            wpool = ctx.enter_context(tc.tile_pool(name="wp", bufs=WBUFS))
            pspool = ctx.enter_context(
                tc.tile_pool(name="ps", bufs=8, space="PSUM"))
            opool = ctx.enter_context(tc.tile_pool(name="op", bufs=8))
            bpool = ctx.enter_context(tc.tile_pool(name="bp", bufs=1))

            bias_sb = bpool.tile([128, N_PAIRS], f32, tag="bias",
                                 name="bias_sb")
            nc.scalar.dma_start(bias_sb[:], bias_d[:])

            # Weights: re-streamed per pass on the sync queue (which has
            # no compute-waiting instructions), in consumption order. The
            # small pool paces the stream at the DESCRIPTOR level (group
            # g's DMA waits until group g-WBUFS is consumed), so x gets
            # bandwidth priority during the prologue automatically.
            wg_tiles = {}

            def ensure_wgroup(s, g):
                if (s, g) not in wg_tiles:
                    t = wpool.tile([128, WGRP * BLK], sdt, tag="wg",
                                   name=f"wg{s}_{g}")
                    nc.sync.dma_start(t[:], wg_d[g, :, :])
                    wg_tiles[(s, g)] = t
                return wg_tiles[(s, g)]

            # x slabs: all chunks eagerly streamed in consumption order,
            # alternating between two DMA queues for bandwidth. Slab 0
            # uses finer chunks so compute can start sooner.
            xs = {}
            qtog = 0
            for s in range(n_sl):
                nch = len(ch0) if s == 0 else n_chunks
                for cb in range(nch):
                    if s == 0:
                        w_ = ch0[cb][1]
                        src = xt0a_d[cb, :, :] if cb < n_fine \
                            else xt0b_d[cb - n_fine, :, :]
                    else:
                        w_ = XCH
                        src = xt_d[s - 1, cb, :, :]
                    t = xpool.tile([128, w_ * TOK], sdt,
                                   tag=f"x{s}_{cb}", name=f"x{s}_{cb}")
                    eng = nc.scalar if qtog % 2 == 0 else nc.gpsimd
                    qtog += 1
                    eng.dma_start(t[:], src)
                    xs[(s, cb)] = t

            def x_ap(c_pos, ki, s):
                if s == 0:
                    if c_pos < n_fine * XCH0:
                        cb, wi = divmod(c_pos, XCH0)
                    else:
                        b, wi = divmod(c_pos - n_fine * XCH0, XCH)
                        cb = n_fine + b
                else:
                    cb, wi = divmod(c_pos, XCH)
                t = xs[(s, cb)]
                return t[ki * 64:(ki + 1) * 64, wi * TOK:(wi + 1) * TOK]

            # HAM pre-warm: ~4.3us of dummy PE work starting right after
            # the bias DMA, so real matmuls begin at 2.4GHz. Results land
            # in a scratch psum tile whose later reuse starts fresh.
            scr = bpool.tile([128, TOK], sdt, tag="scr", name="scr")
            nc.vector.memset(scr[:], 0.0)
            ps_scr = pspool.tile([128, TOK], f32, tag="ps", name="ps_scr")
            for i in range(10):
                nc.tensor.matmul(
                    ps_scr[0:64, :], scr[0:64, 0:64], scr[0:64, :],
                    start=True, stop=True, tile_position=(0, 0),
                    skip_group_check=True)

            def evict(p, s, pt):
                osb = opool.tile([128, TOK], sdt, tag="osb",
                                 name=f"o{s}_{p}")
                if pt is not None:
                    if all(v > 0 for v in nmm[p].values()):
                        nc.scalar.activation(
                            osb[:], pt[0][:],
                            mybir.ActivationFunctionType.Identity,
                            bias=bias_sb[:, p:p + 1], scale=1.0)
                        nc.vector.tensor_add(osb[:], osb[:], pt[1][:])
                    else:
                        for mi in (0, 1):
                            oh = osb[mi * 64:(mi + 1) * 64, :]
                            bh = bias_sb[mi * 64:(mi + 1) * 64, p:p + 1]
                            srcs = [pt[ki][mi * 64:(mi + 1) * 64, :]
                                    for ki in (0, 1)
                                    if nmm[p][(ki, mi)] > 0]
                            if not srcs:
                                nc.vector.memset(oh, 0.0)
                                nc.vector.tensor_scalar_add(oh, oh, bh)
                            else:
                                nc.scalar.activation(
                                    oh, srcs[0],
                                    mybir.ActivationFunctionType.Identity,
                                    bias=bh, scale=1.0)
                                if len(srcs) > 1:
                                    nc.vector.tensor_add(oh, oh, srcs[1])
                else:
                    nc.vector.memset(osb[:], 0.0)
                    nc.vector.tensor_scalar_add(osb[:], osb[:],
                                                bias_sb[:, p:p + 1])
                nc.sync.dma_start(
                    yt_d[p * 128:(p + 1) * 128, s * TOK:(s + 1) * TOK],
                    osb[:])

            for s in range(n_sl):
                psum = {}
                started = {}
                for ev in _weave(pairs):
                    if ev[0] == "mm":
                        _, p, (c_pos, c, mi, w, ki) = ev
                        if p not in psum:
                            psum[p] = [
                                pspool.tile([128, TOK], f32, tag="ps",
                                            name=f"ps{s}_{p}_{k}")
                                for k in range(2)]
                            started[p] = {qd: 0 for qd in QUADS}
                        g, j = divmod(slot[w], WGRP)
                        wt = ensure_wgroup(s, g)
                        lhsT = wt[ki * 64:(ki + 1) * 64,
                                  j * BLK:(j + 1) * BLK]
                        started[p][(ki, mi)] += 1
                        first = started[p][(ki, mi)] == 1
                        last = started[p][(ki, mi)] == nmm[p][(ki, mi)]
                        nc.tensor.matmul(
                            psum[p][ki][mi * 64:(mi + 1) * 64, :],
                            lhsT, x_ap(c_pos, ki, s),
                            start=first, stop=last,
                            tile_position=(ki * 64, mi * 64),
                            skip_group_check=True,
                        )
                    else:
                        _, p = ev
                        evict(p, s, psum.pop(p, None))
    nc.compile()
    return nc


def kernel(x, weight, bias, row_idx, col_idx):
    x = np.asarray(x, dtype=np.float32)
    weight = np.asarray(weight, dtype=np.float32)
    bias = np.asarray(bias, dtype=np.float32)
    row_idx = np.asarray(row_idx)
    col_idx = np.asarray(col_idx)
    ntok_total = x.shape[0]
    assert ntok_total % N_CORES == 0
    ntok = ntok_total // N_CORES
    n_sl = ntok // TOK
    n_chunks = (IN_BLK // 2 + XCH - 1) // XCH

    half, pos = _assign_cols(row_idx, col_idx)
    pairs = _build_pairs(row_idx, col_idx, half, pos)
    slot, n_groups = _assign_slots(pairs)
    wgrp, bias_pk = _pack_host_arrays(weight, bias, pairs, slot, n_groups)
    nc = _build_kernel(pairs, slot, n_groups, ntok)

    # x packing: perm of col-blocks by (half, pos), then slab-major
    perm = sorted(range(IN_BLK), key=lambda c: (half[c], pos[c]))
    ch0 = _slab0_chunks()
    n_fine = N_FINE

    def pack_range(xt4, s, p0, w):
        sl = xt4[:, p0:p0 + w, :, s * TOK:(s + 1) * TOK]  # [2,w,64,TOK]
        return sl.transpose(0, 2, 1, 3).reshape(128, w * TOK)

    def pack_slab(xt4, s, xchw, nch):
        sl = xt4[:, :, :, s * TOK:(s + 1) * TOK]       # [2,32,64,TOK]
        sl = sl.reshape(2, nch, xchw, BLK, TOK)
        return sl.transpose(1, 0, 3, 2, 4).reshape(nch, 128, xchw * TOK)

    in_maps = []
    for cid in range(N_CORES):
        xt = np.ascontiguousarray(
            x[cid * ntok:(cid + 1) * ntok].T).astype(BF16)
        xt3 = xt.reshape(IN_BLK, BLK, ntok)[perm]      # [64, 64, ntok]
        xt4 = xt3.reshape(2, IN_BLK // 2, BLK, ntok)   # [half,pos,64,ntok]
        xt0a_pk = np.stack([pack_range(xt4, 0, p0, w)
                            for (p0, w) in ch0[:n_fine]])
        xt0b_pk = np.stack([pack_range(xt4, 0, p0, w)
                            for (p0, w) in ch0[n_fine:]])
        xt_pk = np.empty((max(n_sl - 1, 1), n_chunks, 128, XCH * TOK),
                         dtype=BF16)
        for s in range(1, n_sl):
            xt_pk[s - 1] = pack_slab(xt4, s, XCH, n_chunks)
        in_maps.append({"xt0a_pk": np.ascontiguousarray(xt0a_pk),
                        "xt0b_pk": np.ascontiguousarray(xt0b_pk),
                        "xt_pk": np.ascontiguousarray(xt_pk),
                        "wgrp": wgrp, "bias_pk": bias_pk})

    res = run_bass_kernel_spmd(nc, in_maps, core_ids=list(range(N_CORES)))
    y = np.empty((ntok_total, D_OUT), dtype=np.float32)
    for cid in range(N_CORES):
        y[cid * ntok:(cid + 1) * ntok] = \
            res.results[cid]["yt"].T.astype(np.float32)
    return y


# revision 66
# speedup vs baseline: 1.1859x; 1.1859x over previous
"""Block-sparse linear y = x @ W^T + b on 8 TRN2 NeuronCores.

Problem shape (hardcoded): x [8192, 4096] f32, weight [1024, 64, 64] f32
(64x64 blocks), bias [4096] f32, row_idx/col_idx [1024] int32 over a 64x64
block grid.

V11 strategy: data-parallel over tokens (1024/core). y^T = W x^T + b via
64x64 block matmuls packed 4-wide into PE-array quadrants (tile_position),
bf16 inputs with f32 PSUM accumulate. Two 512-token slab passes: pass s
runs entirely on x-slab s; weights stream on their own (sync) queue paced
at consumption by a small tile pool (descriptor-level reuse waits), re-
streamed per pass, so x keeps DMA priority in the prologue. x streams on
two queues (scalar/gpsimd) with fine 256KB chunks for slab 0 so compute
starts early. Each col-block c gets a host-chosen partition half (ki) +
position; halves balance the four (ki, mi=row-parity) quadrant loads and
positions follow pair-discovery order so early pairs complete on early
chunks and the 4-unit PSUM window rolls. Matmuls weave across the 4
quadrants globally with a multi-unit lookahead so chain imbalance never
idles the PE; ~4us of dummy matmuls pre-warm the HAM clock gate. Output
y^T bf16 on the sync queue (fast drain), bias added on eviction via
scalar activation + vector add, f32 cast on host.
"""

from collections import deque
from contextlib import ExitStack

import numpy as np
import ml_dtypes

import concourse.tile as tile
from concourse import bacc, mybir
from concourse.bass_utils import run_bass_kernel_spmd

BLK = 64
OUT_BLK = 64
IN_BLK = 64
D_IN = IN_BLK * BLK    # 4096
D_OUT = OUT_BLK * BLK  # 4096
N_CORES = 8
N_PAIRS = OUT_BLK // 2
TOK = 512              # tokens per pass (slab)
XCH = 8                # positions per x DMA chunk, slabs >= 1 (1MB)
XCH0 = 2               # fine chunks for slab-0 head (256KB): early start
N_FINE = 4             # slab-0 fine chunks (pos 0..XCH0*N_FINE-1); the
                       # rest of slab 0 uses XCH-wide chunks (better BW)
WGRP = 32              # weight slots per DMA group per ki half (512KB)
WBUFS = 4              # weight pool buffers: paces the stream to
                       # consumption via descriptor-level reuse waits
WEAVE_Q = 40           # refill quadrant queues to this many MMs
BF16 = ml_dtypes.bfloat16
QUADS = ((0, 0), (1, 1), (0, 1), (1, 0))


def _assign_cols(row_idx, col_idx):
    """Assign each col-block c a partition half (ki) and position so the
    four (ki, mi=row-parity) quadrant loads are balanced and each half
    holds exactly IN_BLK//2 cols."""
    n_ev = np.zeros(IN_BLK, dtype=np.int64)
    n_od = np.zeros(IN_BLK, dtype=np.int64)
    for r, c in zip(row_idx, col_idx):
        if int(r) % 2 == 0:
            n_ev[int(c)] += 1
        else:
            n_od[int(c)] += 1
    # iterate cols in discovery order (first pair that uses them) so each
    # early pair's cols split evenly across halves -> early units finish
    # on early x chunks; balance quadrant loads as the primary objective
    first_use = {}
    for r, c in zip(row_idx, col_idx):
        p = int(r) // 2
        if int(c) not in first_use or p < first_use[int(c)]:
            first_use[int(c)] = p
    order = sorted(range(IN_BLK),
                   key=lambda c: (first_use.get(c, N_PAIRS),
                                  -(n_ev[c] + n_od[c])))
    cap = IN_BLK // 2
    load = [[0, 0], [0, 0]]  # [ki][mi]
    cnt = [0, 0]
    half = [0] * IN_BLK
    for c in order:
        best, bestcost = None, None
        for k in (0, 1):
            if cnt[k] >= cap:
                continue
            cost = max(load[k][0] + n_ev[c], load[k][1] + n_od[c])
            if bestcost is None or cost < bestcost:
                best, bestcost = k, cost
        half[c] = best
        load[best][0] += n_ev[c]
        load[best][1] += n_od[c]
        cnt[best] += 1
    # positions in discovery order: cols of early pairs get low positions
    # so early units complete on early x chunks and the PSUM window rolls
    pos = [-1] * IN_BLK
    nxt = [0, 0]
    by_pair = [[] for _ in range(N_PAIRS)]
    for r, c in zip(row_idx, col_idx):
        by_pair[int(r) // 2].append(int(c))
    for p in range(N_PAIRS):
        for c in sorted(by_pair[p]):
            if pos[c] < 0:
                pos[c] = nxt[half[c]]
                nxt[half[c]] += 1
    for c in range(IN_BLK):
        if pos[c] < 0:
            pos[c] = nxt[half[c]]
            nxt[half[c]] += 1
    return half, pos


def _build_pairs(row_idx, col_idx, half, pos):
    """pairs[p] = list of (pos, c, mi, w, ki) sorted by pos (consumption
    order within the x stream)."""
    d = {}
    for i in range(len(row_idx)):
        d[(int(row_idx[i]), int(col_idx[i]))] = i
    pairs = [[] for _ in range(N_PAIRS)]
    for (r, c), w in d.items():
        pairs[r // 2].append((pos[c], c, r % 2, w, half[c]))
    for lst in pairs:
        lst.sort()
    return pairs


def _weave(pairs):
    """Yield ('mm', p, block) / ('evict', p) events for one pass, weaving
    MMs across the four quadrants globally. Blocks from a small window of
    units are interleaved so chain tails never idle a quadrant."""
    q = {qd: deque() for qd in QUADS}
    remaining = {}
    nxt = 0
    done_order = deque()

    def admit():
        nonlocal nxt
        p = nxt
        for blk in pairs[p]:
            _, c, mi, w, ki = blk
            q[(ki, mi)].append((p, blk))
        if pairs[p]:
            remaining[p] = len(pairs[p])
        done_order.append(p)
        nxt += 1

    while nxt < N_PAIRS or any(q.values()) or done_order:
        while nxt < N_PAIRS and (
                sum(len(v) for v in q.values()) < WEAVE_Q):
            admit()
        for qd in QUADS:
            if q[qd]:
                p, blk = q[qd].popleft()
                yield ("mm", p, blk)
                remaining[p] -= 1
                if remaining[p] == 0:
                    del remaining[p]
        while done_order and done_order[0] not in remaining:
            yield ("evict", done_order.popleft())


def _assign_slots(pairs):
    """Weight slot per block, in pass-0 emission order so weight groups
    stream in consumption order."""
    slot = {}
    cnt = [0, 0]
    for ev in _weave(pairs):
        if ev[0] != "mm":
            continue
        _, p, (_, c, mi, w, ki) = ev
        slot[w] = cnt[ki]
        cnt[ki] += 1
    n_groups = (max(cnt) + WGRP - 1) // WGRP
    return slot, max(n_groups, 1)


def _pack_host_arrays(weight, bias, pairs, slot, n_groups):
    wgrp = np.zeros((n_groups, 128, WGRP * BLK), dtype=BF16)
    bias_pk = np.zeros((128, N_PAIRS), dtype=np.float32)
    wT = np.ascontiguousarray(
        np.transpose(np.asarray(weight), (0, 2, 1))).astype(BF16)
    for p in range(N_PAIRS):
        bias_pk[:64, p] = bias[(2 * p) * BLK:(2 * p + 1) * BLK]
        bias_pk[64:, p] = bias[(2 * p + 1) * BLK:(2 * p + 2) * BLK]
        for (_, c, mi, w, ki) in pairs[p]:
            g, j = divmod(slot[w], WGRP)
            wgrp[g, ki * 64:(ki + 1) * 64, j * BLK:(j + 1) * BLK] = wT[w]
    return wgrp, bias_pk


def _slab0_chunks():
    """Slab-0 chunk spec: (start_pos, width) — N_FINE fine chunks for the
    earliest positions, then XCH-wide chunks for the bulk."""
    spec = [(i * XCH0, XCH0) for i in range(N_FINE)]
    p0 = N_FINE * XCH0
    while p0 < IN_BLK // 2:
        w = min(XCH, IN_BLK // 2 - p0)
        spec.append((p0, w))
        p0 += w
    return spec


def _build_kernel(pairs, slot, n_groups, ntok):
    assert ntok % TOK == 0
    n_sl = ntok // TOK
    ch0 = _slab0_chunks()
    n_fine = N_FINE
    n_bulk = len(ch0) - n_fine
    n_chunks = (IN_BLK // 2 + XCH - 1) // XCH
    sdt = mybir.dt.bfloat16
    f32 = mybir.dt.float32

    nc = bacc.Bacc("TRN2", target_bir_lowering=False, debug=False)
    xt0a_d = nc.dram_tensor("xt0a_pk", [n_fine, 128, XCH0 * TOK], sdt,
                            kind="ExternalInput").ap()
    xt0b_d = nc.dram_tensor("xt0b_pk", [n_bulk, 128, XCH * TOK], sdt,
                            kind="ExternalInput").ap()
    xt_d = nc.dram_tensor("xt_pk",
                          [max(n_sl - 1, 1), n_chunks, 128, XCH * TOK],
                          sdt, kind="ExternalInput").ap()
    wg_d = nc.dram_tensor("wgrp", [n_groups, 128, WGRP * BLK], sdt,
                          kind="ExternalInput").ap()
    bias_d = nc.dram_tensor("bias_pk", [128, N_PAIRS], f32,
                            kind="ExternalInput").ap()
    yt_d = nc.dram_tensor("yt", [D_OUT, ntok], sdt,
                          kind="ExternalOutput").ap()

    nmm = []
    for p in range(N_PAIRS):
        m = {qd: 0 for qd in QUADS}
        for (_, c, mi, w, ki) in pairs[p]:
            m[(ki, mi)] += 1
        nmm.append(m)

    with tile.TileContext(nc) as tc:
        with ExitStack() as ctx:
            xpool = ctx.enter_context(tc.tile_pool(name="xp", bufs=1))
            wpool = ctx.enter_context(tc.tile_pool(name="wp", bufs=WBUFS))
            pspool = ctx.enter_context(
                tc.tile_pool(name="ps", bufs=8, space="PSUM"))
            opool = ctx.enter_context(tc.tile_pool(name="op", bufs=8))
            bpool = ctx.enter_context(tc.tile_pool(name="bp", bufs=1))

            bias_sb = bpool.tile([128, N_PAIRS], f32, tag="bias",
                                 name="bias_sb")
            nc.scalar.dma_start(bias_sb[:], bias_d[:])

            # Weights: re-streamed per pass on the sync queue (which has
            # no compute-waiting instructions), in consumption order. The
            # small pool paces the stream at the DESCRIPTOR level (group
            # g's DMA waits until group g-WBUFS is consumed), so x gets
            # bandwidth priority during the prologue automatically.
            wg_tiles = {}

            def ensure_wgroup(s, g):
                if (s, g) not in wg_tiles:
                    t = wpool.tile([128, WGRP * BLK], sdt, tag="wg",
                                   name=f"wg{s}_{g}")
                    nc.sync.dma_start(t[:], wg_d[g, :, :])
                    wg_tiles[(s, g)] = t
                return wg_tiles[(s, g)]

            # x slabs: all chunks eagerly streamed in consumption order,
            # alternating between two DMA queues for bandwidth. Slab 0
            # uses finer chunks so compute can start sooner.
            xs = {}
            qtog = 0
            for s in range(n_sl):
                nch = len(ch0) if s == 0 else n_chunks
                for cb in range(nch):
                    if s == 0:
                        w_ = ch0[cb][1]
                        src = xt0a_d[cb, :, :] if cb < n_fine \
                            else xt0b_d[cb - n_fine, :, :]
                    else:
                        w_ = XCH
                        src = xt_d[s - 1, cb, :, :]
                    t = xpool.tile([128, w_ * TOK], sdt,
                                   tag=f"x{s}_{cb}", name=f"x{s}_{cb}")
                    eng = nc.scalar if qtog % 2 == 0 else nc.gpsimd
                    qtog += 1
                    eng.dma_start(t[:], src)
                    xs[(s, cb)] = t

            def x_ap(c_pos, ki, s):
                if s == 0:
                    if c_pos < n_fine * XCH0:
                        cb, wi = divmod(c_pos, XCH0)
                    else:
                        b, wi = divmod(c_pos - n_fine * XCH0, XCH)
                        cb = n_fine + b
                else:
                    cb, wi = divmod(c_pos, XCH)
                t = xs[(s, cb)]
                return t[ki * 64:(ki + 1) * 64, wi * TOK:(wi + 1) * TOK]

            # HAM pre-warm: ~4.3us of dummy PE work starting right after
            # the bias DMA, so real matmuls begin at 2.4GHz. Results land
            # in a scratch psum tile whose later reuse starts fresh.
            scr = bpool.tile([128, TOK], sdt, tag="scr", name="scr")
            nc.vector.memset(scr[:], 0.0)
            ps_scr = pspool.tile([128, TOK], f32, tag="ps", name="ps_scr")
            for i in range(10):
                nc.tensor.matmul(
                    ps_scr[0:64, :], scr[0:64, 0:64], scr[0:64, :],
                    start=True, stop=True, tile_position=(0, 0),
                    skip_group_check=True)

            def evict(p, s, pt):
                osb = opool.tile([128, TOK], sdt, tag="osb",
                                 name=f"o{s}_{p}")
                if pt is not None:
                    if all(v > 0 for v in nmm[p].values()):
                        nc.scalar.activation(
                            osb[:], pt[0][:],
                            mybir.ActivationFunctionType.Identity,
                            bias=bias_sb[:, p:p + 1], scale=1.0)
                        nc.vector.tensor_add(osb[:], osb[:], pt[1][:])
                    else:
                        for mi in (0, 1):
                            oh = osb[mi * 64:(mi + 1) * 64, :]
                            bh = bias_sb[mi * 64:(mi + 1) * 64, p:p + 1]
                            srcs = [pt[ki][mi * 64:(mi + 1) * 64, :]
                                    for ki in (0, 1)
                                    if nmm[p][(ki, mi)] > 0]
                            if not srcs:
                                nc.vector.memset(oh, 0.0)
                                nc.vector.tensor_scalar_add(oh, oh, bh)
                            else:
                                nc.scalar.activation(
                                    oh, srcs[0],
                                    mybir.ActivationFunctionType.Identity,
                                    bias=bh, scale=1.0)
                                if len(srcs) > 1:
                                    nc.vector.tensor_add(oh, oh, srcs[1])
                else:
                    nc.vector.memset(osb[:], 0.0)
                    nc.vector.tensor_scalar_add(osb[:], osb[:],
                                                bias_sb[:, p:p + 1])
                nc.sync.dma_start(
                    yt_d[p * 128:(p + 1) * 128, s * TOK:(s + 1) * TOK],
                    osb[:])

            for s in range(n_sl):
                psum = {}
                started = {}
                for ev in _weave(pairs):
                    if ev[0] == "mm":
                        _, p, (c_pos, c, mi, w, ki) = ev
                        if p not in psum:
                            psum[p] = [
                                pspool.tile([128, TOK], f32, tag="ps",
                                            name=f"ps{s}_{p}_{k}")
                                for k in range(2)]
                            started[p] = {qd: 0 for qd in QUADS}
                        g, j = divmod(slot[w], WGRP)
                        wt = ensure_wgroup(s, g)
                        lhsT = wt[ki * 64:(ki + 1) * 64,
                                  j * BLK:(j + 1) * BLK]
                        started[p][(ki, mi)] += 1
                        first = started[p][(ki, mi)] == 1
                        last = started[p][(ki, mi)] == nmm[p][(ki, mi)]
                        nc.tensor.matmul(
                            psum[p][ki][mi * 64:(mi + 1) * 64, :],
                            lhsT, x_ap(c_pos, ki, s),
                            start=first, stop=last,
                            tile_position=(ki * 64, mi * 64),
                            skip_group_check=True,
                        )
                    else:
                        _, p = ev
                        evict(p, s, psum.pop(p, None))
    nc.compile()
    return nc


def kernel(x, weight, bias, row_idx, col_idx):
    x = np.asarray(x, dtype=np.float32)
    weight = np.asarray(weight, dtype=np.float32)
    bias = np.asarray(bias, dtype=np.float32)
    row_idx = np.asarray(row_idx)
    col_idx = np.asarray(col_idx)
    ntok_total = x.shape[0]
    assert ntok_total % N_CORES == 0
    ntok = ntok_total // N_CORES
    n_sl = ntok // TOK
    n_chunks = (IN_BLK // 2 + XCH - 1) // XCH

    half, pos = _assign_cols(row_idx, col_idx)
    pairs = _build_pairs(row_idx, col_idx, half, pos)
    slot, n_groups = _assign_slots(pairs)
    wgrp, bias_pk = _pack_host_arrays(weight, bias, pairs, slot, n_groups)
    nc = _build_kernel(pairs, slot, n_groups, ntok)

    # x packing: perm of col-blocks by (half, pos), then slab-major
    perm = sorted(range(IN_BLK), key=lambda c: (half[c], pos[c]))
    ch0 = _slab0_chunks()
    n_fine = N_FINE

    def pack_range(xt4, s, p0, w):
        sl = xt4[:, p0:p0 + w, :, s * TOK:(s + 1) * TOK]  # [2,w,64,TOK]
        return sl.transpose(0, 2, 1, 3).reshape(128, w * TOK)

    def pack_slab(xt4, s, xchw, nch):
        sl = xt4[:, :, :, s * TOK:(s + 1) * TOK]       # [2,32,64,TOK]
        sl = sl.reshape(2, nch, xchw, BLK, TOK)
        return sl.transpose(1, 0, 3, 2, 4).reshape(nch, 128, xchw * TOK)

    in_maps = []
    for cid in range(N_CORES):
        xt = np.ascontiguousarray(
            x[cid * ntok:(cid + 1) * ntok].T).astype(BF16)
        xt3 = xt.reshape(IN_BLK, BLK, ntok)[perm]      # [64, 64, ntok]
        xt4 = xt3.reshape(2, IN_BLK // 2, BLK, ntok)   # [half,pos,64,ntok]
        xt0a_pk = np.stack([pack_range(xt4, 0, p0, w)
                            for (p0, w) in ch0[:n_fine]])
        xt0b_pk = np.stack([pack_range(xt4, 0, p0, w)
                            for (p0, w) in ch0[n_fine:]])
        xt_pk = np.empty((max(n_sl - 1, 1), n_chunks, 128, XCH * TOK),
                         dtype=BF16)
        for s in range(1, n_sl):
            xt_pk[s - 1] = pack_slab(xt4, s, XCH, n_chunks)
        in_maps.append({"xt0a_pk": np.ascontiguousarray(xt0a_pk),
                        "xt0b_pk": np.ascontiguousarray(xt0b_pk),
                        "xt_pk": np.ascontiguousarray(xt_pk),
                        "wgrp": wgrp, "bias_pk": bias_pk})

    res = run_bass_kernel_spmd(nc, in_maps, core_ids=list(range(N_CORES)))
    y = np.empty((ntok_total, D_OUT), dtype=np.float32)
    for cid in range(N_CORES):
        y[cid * ntok:(cid + 1) * ntok] = \
            res.results[cid]["yt"].T.astype(np.float32)
    return y
